# revision 10
# baseline (speedup 1.0000x reference)
"""Trainium2 Bass kernel for nn_CIG_SA_attention.

Data-parallel over batch: 16 batches -> 8 cores x 2.
Per-batch algorithm (no per-(b,c) data reshuffling): q/k/v live in the 5-map
basis G = [sum_x, max_x, sum_y, max_y, ones]:
  L[c,h,g]   = sum_m coefL[m,c] * Gram_m[h,g],  Gram via 25 PE matmuls of G^T maps
  norms      via d_m[h] = sum_w GT_i[w,h]*GT_j[w,h]  (15 sym pairs)
  S' = softmax-normalized exp(L*rq*rk); M_n = sum_c (w_du*Av)[c,n] S'_c  (PE)
  du_pre = sum_n M_n^T @ G_n (PE, PSUM-accumulated); out = x * sigmoid(du_pre+b).
"""
import numpy as np
from contextlib import ExitStack

import concourse.bass as bass
import concourse.bacc as bacc
import concourse.tile as tile
from concourse import mybir
from concourse.bass_utils import run_bass_kernel_spmd

B, C, H, W = 16, 128, 128, 128
NCORES = 8
BPC = B // NCORES
PIX = H * W
NCH = 32                    # 512-col chunks per batch
F32 = mybir.dt.float32
AX = mybir.AxisListType
OP = mybir.AluOpType
AF = mybir.ActivationFunctionType
PAIRS = [(i, j) for i in range(5) for j in range(i, 5)]


def build_kernel():
    nc = bacc.Bacc("TRN2", target_bir_lowering=False, debug=False, num_devices=NCORES)
    x_d = nc.dram_tensor("x", [BPC, C, PIX], F32, kind="ExternalInput")
    wcwT_d = nc.dram_tensor("wcwT", [C, C], F32, kind="ExternalInput")
    bcw_d = nc.dram_tensor("bcw", [C, 1], F32, kind="ExternalInput")
    coefL_d = nc.dram_tensor("coefL", [25, C], F32, kind="ExternalInput")
    coefQQ_d = nc.dram_tensor("coefQQ", [15, C], F32, kind="ExternalInput")
    coefKK_d = nc.dram_tensor("coefKK", [15, C], F32, kind="ExternalInput")
    cvdu_d = nc.dram_tensor("cvdu", [C, 5], F32, kind="ExternalInput")
    ident_d = nc.dram_tensor("ident", [128, 128], F32, kind="ExternalInput")
    ones_d = nc.dram_tensor("ones", [128, 128], F32, kind="ExternalInput")
    bdu_d = nc.dram_tensor("bdu", [C, 1], F32, kind="ExternalInput")
    out_d = nc.dram_tensor("out", [BPC, C, PIX], F32, kind="ExternalOutput")

    with tile.TileContext(nc) as tc, ExitStack() as ctx:
        cp = ctx.enter_context(tc.tile_pool(name="consts", bufs=1))
        wcwT = cp.tile([C, C], F32)
        bcw = cp.tile([C, 1], F32)
        coefL = cp.tile([25, C], F32)
        coefQQ = cp.tile([15, C], F32)
        coefKK = cp.tile([15, C], F32)
        cvdu = cp.tile([C, 5], F32)
        ident = cp.tile([128, 128], F32)
        ones = cp.tile([128, 128], F32)
        bdu = cp.tile([C, 1], F32)
        for t, d in [(wcwT, wcwT_d), (bcw, bcw_d), (coefL, coefL_d),
                     (coefQQ, coefQQ_d), (coefKK, coefKK_d), (cvdu, cvdu_d),
                     (ident, ident_d), (ones, ones_d), (bdu, bdu_d)]:
            nc.sync.dma_start(t[:], d[:])

        xpool = ctx.enter_context(tc.tile_pool(name="xp", bufs=1))
        gfpool = ctx.enter_context(tc.tile_pool(name="gfp", bufs=1))
        gtpool = ctx.enter_context(tc.tile_pool(name="gtp", bufs=1))
        smp = ctx.enter_context(tc.tile_pool(name="smp", bufs=1))
        outp = ctx.enter_context(tc.tile_pool(name="outp", bufs=4))

        for b in range(BPC):
            xb = xpool.tile([C, NCH, 512], F32, tag="xb")
            nc.sync.dma_start(xb[:], x_d[b, :, :].rearrange("c (n f) -> c n f", f=512))

            GT = gtpool.tile([128, 5, 128], F32, tag="GT")   # [w, map, h]
            Gn = gtpool.tile([128, 5, 128], F32, tag="Gn")   # [g, map, w]
            nc.vector.tensor_copy(GT[:, 4, :], ones[:])

            # ---- gate, y, per-pixel channel reductions -> G^T maps ----
            with tc.tile_pool(name="s2sb", bufs=3) as sbp, \
                 tc.tile_pool(name="s2ps", bufs=2, space=bass.MemorySpace.PSUM) as psp:
                for n in range(NCH):
                    x4 = xb[:, n, :].rearrange("c (a f) -> c a f", f=128)
                    pz = psp.tile([C, 512], F32, tag="pz")
                    nc.tensor.matmul(pz[:], wcwT[:], xb[:, n, :], start=True, stop=True)
                    sig = sbp.tile([C, 4, 128], F32, tag="sig")
                    nc.scalar.activation(sig[:], pz[:].rearrange("c (a f) -> c a f", f=128),
                                         AF.Sigmoid, bias=bcw[:, 0:1])
                    ych = sbp.tile([C, 4, 128], F32, tag="ych")
                    nc.vector.scalar_tensor_tensor(ych[:], sig[:], 0.5, x4,
                                                   op0=OP.max, op1=OP.mult)
                    ptx = psp.tile([128, 4, 128], F32, tag="ptx")
                    pty = psp.tile([128, 4, 128], F32, tag="pty")
                    for j in range(4):
                        nc.tensor.transpose(ptx[:, j, :], x4[:, j, :], ident[:])
                        nc.tensor.transpose(pty[:, j, :], ych[:, j, :], ident[:])
                    nc.vector.tensor_reduce(GT[:, 0, 4*n:4*n+4], ptx[:], axis=AX.X, op=OP.add)
                    nc.vector.tensor_reduce(GT[:, 1, 4*n:4*n+4], ptx[:], axis=AX.X, op=OP.max)
                    nc.vector.tensor_reduce(GT[:, 2, 4*n:4*n+4], pty[:], axis=AX.X, op=OP.add)
                    nc.vector.tensor_reduce(GT[:, 3, 4*n:4*n+4], pty[:], axis=AX.X, op=OP.max)

            # ---- Gram maps -> gflat [25, pix]; natural G maps; norm factors ----
            gflat = gfpool.tile([25, NCH, 512], F32, tag="gflat")
            rq = smp.tile([C, 128], F32, tag="rq")
            rk = smp.tile([C, 128], F32, tag="rk")
            with tc.tile_pool(name="gsb", bufs=4) as gsb, \
                 tc.tile_pool(name="gps", bufs=2, space=bass.MemorySpace.PSUM) as gps, \
                 tc.tile_pool(name="gps1", bufs=1, space=bass.MemorySpace.PSUM) as gps1, \
                 tc.tile_pool(name="gsb1", bufs=1) as gsb1:
                for i in range(5):
                    for j in range(5):
                        m = 5 * i + j
                        pg = gps.tile([128, 128], F32, tag="pg")
                        nc.tensor.matmul(pg[:], GT[:, i, :], GT[:, j, :], start=True, stop=True)
                        gtl = gsb.tile([128, 128], F32, tag="gtl")
                        nc.scalar.copy(gtl[:], pg[:])
                        nc.sync.dma_start(gflat[m:m+1, :, :].rearrange("p n f -> p (n f)"),
                                          gtl[:])
                    pgn = gps.tile([128, 128], F32, tag="pgn")
                    nc.tensor.transpose(pgn[:], GT[:, i, :], ident[:])
                    nc.scalar.copy(Gn[:, i, :], pgn[:])

                # d_m[h] = sum_w GT_i * GT_j (15 pairs) via ones-matmul dup rows
                P = gsb1.tile([128, 15, 128], F32, tag="P")
                for mi, (i, j) in enumerate(PAIRS):
                    nc.vector.tensor_mul(P[:, mi, :], GT[:, i, :], GT[:, j, :])
                Pf = P[:].rearrange("w m h -> w (m h)")
                drow = gsb1.tile([1, 15, 128], F32, tag="drow")
                for q in range(4):
                    pd = gps.tile([128, 480], F32, tag="pg")
                    nc.tensor.matmul(pd[:], ones[:], Pf[:, 480*q:480*(q+1)],
                                     start=True, stop=True)
                    nc.scalar.copy(drow[:].rearrange("o m h -> o (m h)")[:, 480*q:480*(q+1)],
                                   pd[0:1, :])
                d15 = gsb1.tile([15, 128], F32, tag="d15")
                nc.sync.dma_start(d15[:],
                                  drow[:].rearrange("o m h -> o (m h)"))
                pnq = gps1.tile([C, 128], F32, tag="pnq")
                nc.tensor.matmul(pnq[:], coefQQ[:], d15[:], start=True, stop=True)
                sq = gsb.tile([C, 128], F32, tag="sq")
                nc.scalar.activation(sq[:], pnq[:], AF.Sqrt)
                nc.vector.reciprocal(rq[:], sq[:])
                pnk = gps1.tile([C, 128], F32, tag="pnk")
                nc.tensor.matmul(pnk[:], coefKK[:], d15[:], start=True, stop=True)
                sk = gsb.tile([C, 128], F32, tag="sk")
                nc.scalar.activation(sk[:], pnk[:], AF.Sqrt)
                nc.vector.reciprocal(rk[:], sk[:])

            # ---- attention chunks: L -> exp -> softmax-normalize -> M ----
            Mt = gtpool.tile([128, 5, 128], F32, tag="Mt")   # [h, map, g]
            with tc.tile_pool(name="asb", bufs=3) as asb, \
                 tc.tile_pool(name="msb", bufs=2) as msb, \
                 tc.tile_pool(name="aps", bufs=2, space=bass.MemorySpace.PSUM) as aps:
                for n in range(NCH):
                    pL = aps.tile([C, 512], F32, tag="pL")
                    nc.tensor.matmul(pL[:], coefL[:], gflat[:, n, :], start=True, stop=True)
                    pL4 = pL[:].rearrange("c (a f) -> c a f", f=128)
                    Ls = asb.tile([C, 4, 128], F32, tag="Ls")
                    for hh in range(4):
                        nc.vector.scalar_tensor_tensor(
                            Ls[:, hh, :], pL4[:, hh, :], rq[:, 4*n+hh:4*n+hh+1],
                            rk[:, :], op0=OP.mult, op1=OP.mult)
                    Se = asb.tile([C, 4, 128], F32, tag="Se")
                    nc.scalar.activation(Se[:], Ls[:], AF.Exp)
                    rc = asb.tile([C, 4], F32, tag="rc")
                    nc.vector.tensor_reduce(rc[:], Se[:], axis=AX.X, op=OP.add)
                    rho = asb.tile([C, 4], F32, tag="rho")
                    nc.vector.reciprocal(rho[:], rc[:])
                    Sp = asb.tile([C, 4, 128], F32, tag="Sp")
                    for hh in range(4):
                        nc.vector.tensor_scalar_mul(Sp[:, hh, :], Se[:, hh, :],
                                                    rho[:, hh:hh+1])
                    pM = aps.tile([5, 512], F32, tag="pM")
                    nc.tensor.matmul(pM[:], cvdu[:], Sp[:].rearrange("c a f -> c (a f)"),
                                     start=True, stop=True)
                    if n % 4 == 0:
                        m4 = msb.tile([5, 16, 128], F32, tag="m4")
                    nc.scalar.copy(m4[:, 4*(n % 4):4*(n % 4)+4, :],
                                   pM[:].rearrange("p (a f) -> p a f", f=128))
                    if n % 4 == 3:
                        for nn in range(5):
                            nc.sync.dma_start(Mt[16*(n//4):16*(n//4)+16, nn, :],
                                              m4[nn:nn+1, :, :])

            # ---- du_pre = sum_n M_n^T @ G_n ; du = sigmoid ; out = x*du ----
            with tc.tile_pool(name="dsb", bufs=2) as dsb, \
                 tc.tile_pool(name="dps", bufs=2, space=bass.MemorySpace.PSUM) as dps:
                MT = dsb.tile([128, 5, 128], F32, tag="MT")  # [g, map, h]
                for nn in range(5):
                    pmt = dps.tile([128, 128], F32, tag="pmt")
                    nc.tensor.transpose(pmt[:], Mt[:, nn, :], ident[:])
                    nc.scalar.copy(MT[:, nn, :], pmt[:])
                pdu = dps.tile([128, 128], F32, tag="pdu")
                for nn in range(5):
                    nc.tensor.matmul(pdu[:], MT[:, nn, :], Gn[:, nn, :],
                                     start=(nn == 0), stop=(nn == 4))
                du = dsb.tile([128, 128], F32, tag="du")
                nc.scalar.activation(du[:], pdu[:], AF.Sigmoid, bias=bdu[:, 0:1])
                durow = gfpool.tile([1, PIX], F32, tag="gflat")
                nc.sync.dma_start(durow[:], du[:])
                for n in range(NCH):
                    pbc = dps.tile([128, 512], F32, tag="pbc")
                    nc.tensor.matmul(pbc[:], ones[0:1, :],
                                     durow[0:1, 512*n:512*(n+1)],
                                     start=True, stop=True)
                    oc = outp.tile([C, 512], F32, tag="oc")
                    nc.vector.tensor_mul(oc[:], xb[:, n, :], pbc[:])
                    nc.sync.dma_start(out_d[b, :, 512*n:512*(n+1)], oc[:])
    nc.compile()
    return nc


_NC = None


def kernel(x, w_cw, b_cw, w_qkv, b_qkv, w_du, b_du):
    global _NC
    x = np.asarray(x, np.float32)
    w_cw = np.asarray(w_cw, np.float32)
    b_cw = np.asarray(b_cw, np.float32)
    w_qkv = np.asarray(w_qkv, np.float32)
    b_qkv = np.asarray(b_qkv, np.float32)
    w_du = np.asarray(w_du, np.float32)
    b_du = np.asarray(b_du, np.float32)

    wq, wk, wv = w_qkv[0:C], w_qkv[C:2*C], w_qkv[2*C:3*C]
    bq, bk, bv = b_qkv[0:C], b_qkv[C:2*C], b_qkv[2*C:3*C]

    def amat(w, bias):
        return np.stack([w[:, 0]/C, w[:, 1], w[:, 2]/C, w[:, 3], bias], axis=1)

    Aq, Ak, Av = amat(wq, bq), amat(wk, bk), amat(wv, bv)
    coefL = np.einsum('ci,cj->ijc', Aq, Ak).reshape(25, C).astype(np.float32)
    coefQQ = np.stack([Aq[:, i]*Aq[:, j]*(1.0 if i == j else 2.0)
                       for i, j in PAIRS]).astype(np.float32)
    coefKK = np.stack([Ak[:, i]*Ak[:, j]*(1.0 if i == j else 2.0)
                       for i, j in PAIRS]).astype(np.float32)
    cvdu = (Av * w_du[0][:, None]).astype(np.float32)

    common = {
        "wcwT": np.ascontiguousarray(w_cw.T),
        "bcw": np.ascontiguousarray(b_cw[:, None]),
        "coefL": coefL, "coefQQ": coefQQ, "coefKK": coefKK, "cvdu": cvdu,
        "ident": np.eye(128, dtype=np.float32),
        "ones": np.ones((128, 128), np.float32),
        "bdu": np.full((C, 1), b_du[0], np.float32),
    }
    xs = x.reshape(NCORES, BPC, C, PIX)
    in_maps = [dict(common, x=np.ascontiguousarray(xs[k])) for k in range(NCORES)]

    if _NC is None:
        _NC = build_kernel()
    globals()["_LAST_IN_MAPS"] = in_maps
    res = run_bass_kernel_spmd(_NC, in_maps, core_ids=list(range(NCORES)))
    out = np.concatenate([r["out"][None] for r in res.results], axis=0)
    return out.reshape(B, C, H, W)


if __name__ == "__main__":
    import reference as R
    inp = R.setup_inputs()
    inp = {k: np.asarray(v) for k, v in inp.items()}
    got = kernel(**inp)
    exp = np.asarray(R.reference(**inp))
    err = np.abs(got - exp).max() / np.abs(exp).max()
    print("scaled absmax err:", err)


# revision 17
# speedup vs baseline: 2.8136x; 2.8136x over previous
"""Trainium2 Bass kernel for nn_CIG_SA_attention.

Data-parallel over batch: 16 batches -> 8 cores x 2.
Per-batch algorithm (no per-(b,c) data reshuffling): q/k/v live in the 5-map
basis G = [sum_x, max_x, sum_y, max_y, ones]:
  L[c,h,g]   = sum_m coefL[m,c] * Gram_m[h,g],  Gram via 25 PE matmuls of G^T maps
  norms      via d_m[h] = sum_w GT_i[w,h]*GT_j[w,h]  (15 sym pairs)
  S' = softmax-normalized exp(L*rq*rk); M_n = sum_c (w_du*Av)[c,n] S'_c  (PE)
  du_pre = sum_n M_n^T @ G_n (PE, PSUM-accumulated); out = x * sigmoid(du_pre+b).
"""
import numpy as np
from contextlib import ExitStack

import concourse.bass as bass
import concourse.bacc as bacc
import concourse.tile as tile
from concourse import mybir
from concourse.bass_utils import run_bass_kernel_spmd

B, C, H, W = 16, 128, 128, 128
NCORES = 8
BPC = B // NCORES
PIX = H * W
NCH = 32                    # 512-col chunks per batch
F32 = mybir.dt.float32
AX = mybir.AxisListType
OP = mybir.AluOpType
AF = mybir.ActivationFunctionType
PAIRS = [(i, j) for i in range(5) for j in range(i, 5)]
PAIRS2 = [(i, j) for i in range(5) for j in range(i+1, 5)] + [(i, i) for i in range(5)]


def build_kernel():
    nc = bacc.Bacc("TRN2", target_bir_lowering=False, debug=False, num_devices=NCORES)
    x_d = nc.dram_tensor("x", [BPC, C, PIX], F32, kind="ExternalInput")
    wcwT_d = nc.dram_tensor("wcwT", [C, C], F32, kind="ExternalInput")
    bcw_d = nc.dram_tensor("bcw", [C, 1], F32, kind="ExternalInput")
    coefLA_d = nc.dram_tensor("coefLA", [15, C], F32, kind="ExternalInput")
    coefLB_d = nc.dram_tensor("coefLB", [10, C], F32, kind="ExternalInput")
    coefQQ_d = nc.dram_tensor("coefQQ", [15, C], F32, kind="ExternalInput")
    coefKK_d = nc.dram_tensor("coefKK", [15, C], F32, kind="ExternalInput")
    cvdu_d = nc.dram_tensor("cvdu", [C, 5], F32, kind="ExternalInput")
    ident_d = nc.dram_tensor("ident", [128, 128], F32, kind="ExternalInput")
    ones_d = nc.dram_tensor("ones", [128, 128], F32, kind="ExternalInput")
    bdu_d = nc.dram_tensor("bdu", [C, 1], F32, kind="ExternalInput")
    out_d = nc.dram_tensor("out", [BPC, C, PIX], F32, kind="ExternalOutput")
    gdram = nc.dram_tensor("gdram", [15, 128, 128], F32)

    with tile.TileContext(nc) as tc, ExitStack() as ctx:
        cp = ctx.enter_context(tc.tile_pool(name="consts", bufs=1))
        wcwT = cp.tile([C, C], F32)
        bcw = cp.tile([C, 1], F32)
        coefLA = cp.tile([15, C], F32)
        coefLB = cp.tile([10, C], F32)
        coefQQ = cp.tile([15, C], F32)
        coefKK = cp.tile([15, C], F32)
        cvdu = cp.tile([C, 5], F32)
        ident = cp.tile([128, 128], F32)
        ones = cp.tile([128, 128], F32)
        bdu = cp.tile([C, 1], F32)
        for t, d in [(wcwT, wcwT_d), (bcw, bcw_d), (coefLA, coefLA_d), (coefLB, coefLB_d),
                     (coefQQ, coefQQ_d), (coefKK, coefKK_d), (cvdu, cvdu_d),
                     (ident, ident_d), (ones, ones_d), (bdu, bdu_d)]:
            nc.sync.dma_start(t[:], d[:])

        xpool = ctx.enter_context(tc.tile_pool(name="xp", bufs=1))
        gfpool = ctx.enter_context(tc.tile_pool(name="gfp", bufs=1))
        gtpool = ctx.enter_context(tc.tile_pool(name="gtp", bufs=1))
        smp = ctx.enter_context(tc.tile_pool(name="smp", bufs=1))
        outp = ctx.enter_context(tc.tile_pool(name="outp", bufs=4))

        for b in range(BPC):
            xb = xpool.tile([C, NCH, 512], F32, tag="xb")
            nc.sync.dma_start(xb[:], x_d[b, :, :].rearrange("c (n f) -> c n f", f=512))

            GT = gtpool.tile([128, 5, 128], F32, tag="GT")   # [w, map, h]
            Gn = gtpool.tile([128, 5, 128], F32, tag="Gn")   # [g, map, w]
            nc.vector.tensor_copy(GT[:, 4, :], ones[:])

            # ---- gate, y, per-pixel channel reductions -> G^T maps ----
            with tc.tile_pool(name="s2sb", bufs=3) as sbp, \
                 tc.tile_pool(name="s2ps", bufs=2, space=bass.MemorySpace.PSUM) as psp:
                for n in range(NCH):
                    x4 = xb[:, n, :].rearrange("c (a f) -> c a f", f=128)
                    pz = psp.tile([C, 512], F32, tag="pz")
                    nc.tensor.matmul(pz[:], wcwT[:], xb[:, n, :], start=True, stop=True)
                    sig = sbp.tile([C, 4, 128], F32, tag="sig")
                    nc.scalar.activation(sig[:], pz[:].rearrange("c (a f) -> c a f", f=128),
                                         AF.Sigmoid, bias=bcw[:, 0:1])
                    ych = sbp.tile([C, 4, 128], F32, tag="ych")
                    nc.vector.scalar_tensor_tensor(ych[:], sig[:], 0.5, x4,
                                                   op0=OP.max, op1=OP.mult)
                    ptx = psp.tile([128, 4, 128], F32, tag="ptx")
                    pty = psp.tile([128, 4, 128], F32, tag="pty")
                    for j in range(4):
                        nc.tensor.transpose(ptx[:, j, :], x4[:, j, :], ident[:])
                        nc.tensor.transpose(pty[:, j, :], ych[:, j, :], ident[:])
                    nc.vector.tensor_reduce(GT[:, 0, 4*n:4*n+4], ptx[:], axis=AX.X, op=OP.add)
                    nc.vector.tensor_reduce(GT[:, 1, 4*n:4*n+4], ptx[:], axis=AX.X, op=OP.max)
                    nc.vector.tensor_reduce(GT[:, 2, 4*n:4*n+4], pty[:], axis=AX.X, op=OP.add)
                    nc.vector.tensor_reduce(GT[:, 3, 4*n:4*n+4], pty[:], axis=AX.X, op=OP.max)

            # ---- Gram maps -> gflat [25, pix]; natural G maps; norm factors ----
            gflat = gfpool.tile([15, NCH, 512], F32, tag="gflat")
            rq = smp.tile([C, 128], F32, tag="rq")
            rk = smp.tile([C, 128], F32, tag="rk")
            with tc.tile_pool(name="gsb", bufs=4) as gsb, \
                 tc.tile_pool(name="gps", bufs=2, space=bass.MemorySpace.PSUM) as gps, \
                 tc.tile_pool(name="gps1", bufs=1, space=bass.MemorySpace.PSUM) as gps1, \
                 tc.tile_pool(name="gsb1", bufs=1) as gsb1:
                gsbf = gsb1.tile([128, 15, 128], F32, tag="gsbf")
                for mi, (i, j) in enumerate(PAIRS2):
                    pg = gps.tile([128, 128], F32, tag="pg")
                    nc.tensor.matmul(pg[:], GT[:, i, :], GT[:, j, :], start=True, stop=True)
                    nc.scalar.copy(gsbf[:, mi, :], pg[:])
                nc.sync.dma_start(gdram[:].rearrange("m h g -> h m g"), gsbf[:])
                nc.scalar.dma_start(gflat[:], gdram[:].rearrange("m h (n f) -> m (h n) f", f=512)
                                    if False else gdram[:].rearrange("m h g -> m (h g)")
                                    .rearrange("m (n f) -> m n f", f=512))
                for i in range(5):
                    pgn = gps.tile([128, 128], F32, tag="pgn")
                    nc.tensor.transpose(pgn[:], GT[:, i, :], ident[:])
                    nc.scalar.copy(Gn[:, i, :], pgn[:])

                # d_m[h] = sum_w GT_i * GT_j (15 pairs) via ones-matmul dup rows
                P = gsb1.tile([128, 15, 128], F32, tag="P")
                for mi, (i, j) in enumerate(PAIRS):
                    nc.gpsimd.tensor_mul(P[:, mi, :], GT[:, i, :], GT[:, j, :])
                Pf = P[:].rearrange("w m h -> w (m h)")
                drow = gsb1.tile([1, 15, 128], F32, tag="drow")
                for q in range(4):
                    pd = gps.tile([128, 480], F32, tag="pg")
                    nc.tensor.matmul(pd[:], ones[:], Pf[:, 480*q:480*(q+1)],
                                     start=True, stop=True)
                    nc.scalar.copy(drow[:].rearrange("o m h -> o (m h)")[:, 480*q:480*(q+1)],
                                   pd[0:1, :])
                d15 = gsb1.tile([15, 128], F32, tag="d15")
                nc.sync.dma_start(d15[:],
                                  drow[:].rearrange("o m h -> o (m h)"))
                pnq = gps1.tile([C, 128], F32, tag="pnq")
                nc.tensor.matmul(pnq[:], coefQQ[:], d15[:], start=True, stop=True)
                sq = gsb.tile([C, 128], F32, tag="sq")
                nc.scalar.activation(sq[:], pnq[:], AF.Sqrt)
                nc.vector.reciprocal(rq[:], sq[:])
                pnk = gps1.tile([C, 128], F32, tag="pnk")
                nc.tensor.matmul(pnk[:], coefKK[:], d15[:], start=True, stop=True)
                sk = gsb.tile([C, 128], F32, tag="sk")
                nc.scalar.activation(sk[:], pnk[:], AF.Sqrt)
                nc.vector.reciprocal(rk[:], sk[:])

            # ---- attention chunks: L -> exp -> softmax-normalize -> M ----
            MT = gtpool.tile([128, 5, 128], F32, tag="MT")   # [g, map, h]
            with tc.tile_pool(name="asb", bufs=3) as asb, \
                 tc.tile_pool(name="msb", bufs=2) as msb, \
                 tc.tile_pool(name="aps", bufs=2, space=bass.MemorySpace.PSUM) as aps:
                for n in range(NCH):
                    pL = aps.tile([C, 512], F32, tag="pL")
                    nc.tensor.matmul(pL[:], coefLA[:], gflat[:, n, :], start=True, stop=False)
                    gvT = gflat[:].rearrange("m n f -> m (n f)") \
                                  .rearrange("m (g h) -> m h g", h=128)
                    nc.tensor.matmul(pL[:], coefLB[:], gvT[0:10, 4*n:4*n+4, :],
                                     start=False, stop=True)
                    pL4 = pL[:].rearrange("c (a f) -> c a f", f=128)
                    Ls = asb.tile([C, 4, 128], F32, tag="Ls")
                    for hh in range(4):
                        nc.vector.scalar_tensor_tensor(
                            Ls[:, hh, :], pL4[:, hh, :], rq[:, 4*n+hh:4*n+hh+1],
                            rk[:, :], op0=OP.mult, op1=OP.mult)
                    Se = asb.tile([C, 4, 128], F32, tag="Se")
                    rc = asb.tile([C, 4], F32, tag="rc")
                    for hh in range(4):
                        nc.scalar.activation(Se[:, hh, :], Ls[:, hh, :], AF.Exp,
                                             accum_out=rc[:, hh:hh+1])
                    rho = asb.tile([C, 4], F32, tag="rho")
                    nc.vector.reciprocal(rho[:], rc[:])
                    Sp = asb.tile([C, 4, 128], F32, tag="Sp")
                    for hh in range(4):
                        nc.gpsimd.tensor_scalar_mul(Sp[:, hh, :], Se[:, hh, :],
                                                    rho[:, hh:hh+1])
                    pM = aps.tile([5, 512], F32, tag="pM")
                    nc.tensor.matmul(pM[:], cvdu[:], Sp[:].rearrange("c a f -> c (a f)"),
                                     start=True, stop=True)
                    m4 = msb.tile([5, 4, 128], F32, tag="m4")
                    nc.scalar.copy(m4[:], pM[:].rearrange("p (a f) -> p a f", f=128))
                    for hh in range(4):
                        pmtc = aps.tile([128, 5], F32, tag="pmtc")
                        nc.tensor.transpose(pmtc[:], m4[:, hh, :], ident[0:5, 0:5])
                        nc.vector.tensor_copy(MT[:, :, 4*n+hh], pmtc[:])

            # ---- du_pre = sum_n M_n^T @ G_n ; du = sigmoid ; out = x*du ----
            with tc.tile_pool(name="dsb", bufs=2) as dsb, \
                 tc.tile_pool(name="dps", bufs=2, space=bass.MemorySpace.PSUM) as dps:
                pdu = dps.tile([128, 128], F32, tag="pdu")
                for nn in range(5):
                    nc.tensor.matmul(pdu[:], MT[:, nn, :], Gn[:, nn, :],
                                     start=(nn == 0), stop=(nn == 4))
                du = dsb.tile([128, 128], F32, tag="du")
                nc.scalar.activation(du[:], pdu[:], AF.Sigmoid, bias=bdu[:, 0:1])
                durow = gfpool.tile([1, PIX], F32, tag="gflat")
                nc.gpsimd.dma_start(durow[:], du[:])
                for n in range(NCH):
                    pbc = dps.tile([128, 512], F32, tag="pbc")
                    nc.tensor.matmul(pbc[:], ones[0:1, :],
                                     durow[0:1, 512*n:512*(n+1)],
                                     start=True, stop=True)
                    if n % 4 == 0:
                        oc = outp.tile([C, 4, 512], F32, tag="oc")
                    nc.vector.tensor_mul(oc[:, n % 4, :], xb[:, n, :], pbc[:])
                    if n % 4 == 3:
                        dq = (nc.sync, nc.scalar, nc.gpsimd)[(n//4) % 3]
                        dq.dma_start(out_d[b, :, 2048*(n//4):2048*(n//4 + 1)],
                                     oc[:].rearrange("c a f -> c (a f)"))
    nc.compile()
    return nc


_NC = None


def prep_in_maps(x, w_cw, b_cw, w_qkv, b_qkv, w_du, b_du):
    x = np.asarray(x, np.float32)
    w_cw = np.asarray(w_cw, np.float32)
    b_cw = np.asarray(b_cw, np.float32)
    w_qkv = np.asarray(w_qkv, np.float32)
    b_qkv = np.asarray(b_qkv, np.float32)
    w_du = np.asarray(w_du, np.float32)
    b_du = np.asarray(b_du, np.float32)

    wq, wk, wv = w_qkv[0:C], w_qkv[C:2*C], w_qkv[2*C:3*C]
    bq, bk, bv = b_qkv[0:C], b_qkv[C:2*C], b_qkv[2*C:3*C]

    def amat(w, bias):
        return np.stack([w[:, 0]/C, w[:, 1], w[:, 2]/C, w[:, 3], bias], axis=1)

    Aq, Ak, Av = amat(wq, bq), amat(wk, bk), amat(wv, bv)
    coefLA = np.stack([Aq[:, i]*Ak[:, j] for i, j in PAIRS2]).astype(np.float32)
    coefLB = np.stack([Aq[:, j]*Ak[:, i] for i, j in PAIRS2[:10]]).astype(np.float32)
    coefQQ = np.stack([Aq[:, i]*Aq[:, j]*(1.0 if i == j else 2.0)
                       for i, j in PAIRS]).astype(np.float32)
    coefKK = np.stack([Ak[:, i]*Ak[:, j]*(1.0 if i == j else 2.0)
                       for i, j in PAIRS]).astype(np.float32)
    cvdu = (Av * w_du[0][:, None]).astype(np.float32)

    common = {
        "wcwT": np.ascontiguousarray(w_cw.T),
        "bcw": np.ascontiguousarray(b_cw[:, None]),
        "coefLA": coefLA, "coefLB": coefLB,
        "coefQQ": coefQQ, "coefKK": coefKK, "cvdu": cvdu,
        "ident": np.eye(128, dtype=np.float32),
        "ones": np.ones((128, 128), np.float32),
        "bdu": np.full((C, 1), b_du[0], np.float32),
    }
    xs = x.reshape(NCORES, BPC, C, PIX)
    return [dict(common, x=np.ascontiguousarray(xs[k])) for k in range(NCORES)]


def kernel(x, w_cw, b_cw, w_qkv, b_qkv, w_du, b_du):
    global _NC
    in_maps = prep_in_maps(x, w_cw, b_cw, w_qkv, b_qkv, w_du, b_du)
    if _NC is None:
        _NC = build_kernel()
    globals()["_LAST_IN_MAPS"] = in_maps
    res = run_bass_kernel_spmd(_NC, in_maps, core_ids=list(range(NCORES)))
    out = np.concatenate([r["out"][None] for r in res.results], axis=0)
    return out.reshape(B, C, H, W)


if __name__ == "__main__":
    import reference as R
    inp = R.setup_inputs()
    inp = {k: np.asarray(v) for k, v in inp.items()}
    got = kernel(**inp)
    exp = np.asarray(R.reference(**inp))
    err = np.abs(got - exp).max() / np.abs(exp).max()
    print("scaled absmax err:", err)


# revision 20
# speedup vs baseline: 2.9412x; 1.0454x over previous
"""Trainium2 Bass kernel for nn_CIG_SA_attention.

Data-parallel over batch: 16 batches -> 8 cores x 2.
Per-batch algorithm (no per-(b,c) data reshuffling): q/k/v live in the 5-map
basis G = [sum_x, max_x, sum_y, max_y, ones]:
  L[c,h,g]   = sum_m coefL[m,c] * Gram_m[h,g],  Gram via 25 PE matmuls of G^T maps
  norms      via d_m[h] = sum_w GT_i[w,h]*GT_j[w,h]  (15 sym pairs)
  S' = softmax-normalized exp(L*rq*rk); M_n = sum_c (w_du*Av)[c,n] S'_c  (PE)
  du_pre = sum_n M_n^T @ G_n (PE, PSUM-accumulated); out = x * sigmoid(du_pre+b).
"""
import numpy as np
from contextlib import ExitStack

import concourse.bass as bass
import concourse.bacc as bacc
import concourse.tile as tile
from concourse import mybir
from concourse.bass_utils import run_bass_kernel_spmd

B, C, H, W = 16, 128, 128, 128
NCORES = 8
BPC = B // NCORES
PIX = H * W
NCH = 32                    # 512-col chunks per batch
F32 = mybir.dt.float32
AX = mybir.AxisListType
OP = mybir.AluOpType
AF = mybir.ActivationFunctionType
PAIRS = [(i, j) for i in range(5) for j in range(i, 5)]
PAIRS2 = [(i, j) for i in range(5) for j in range(i+1, 5)] + [(i, i) for i in range(5)]


def build_kernel():
    nc = bacc.Bacc("TRN2", target_bir_lowering=False, debug=False, num_devices=NCORES)
    x_d = nc.dram_tensor("x", [BPC, C, PIX], F32, kind="ExternalInput")
    wcwT_d = nc.dram_tensor("wcwT", [C, C], F32, kind="ExternalInput")
    bcw_d = nc.dram_tensor("bcw", [C, 1], F32, kind="ExternalInput")
    coefLA_d = nc.dram_tensor("coefLA", [15, C], F32, kind="ExternalInput")
    coefLB_d = nc.dram_tensor("coefLB", [10, C], F32, kind="ExternalInput")
    coefQQ_d = nc.dram_tensor("coefQQ", [15, C], F32, kind="ExternalInput")
    coefKK_d = nc.dram_tensor("coefKK", [15, C], F32, kind="ExternalInput")
    cvdu_d = nc.dram_tensor("cvdu", [C, 5], F32, kind="ExternalInput")
    ident_d = nc.dram_tensor("ident", [128, 128], F32, kind="ExternalInput")
    ones_d = nc.dram_tensor("ones", [128, 128], F32, kind="ExternalInput")
    bdu_d = nc.dram_tensor("bdu", [C, 1], F32, kind="ExternalInput")
    out_d = nc.dram_tensor("out", [BPC, C, PIX], F32, kind="ExternalOutput")
    gdram = nc.dram_tensor("gdram", [15, 128, 128], F32)
    mdram = nc.dram_tensor("mdram", [5, 128, 128], F32)

    with tile.TileContext(nc) as tc, ExitStack() as ctx:
        cp = ctx.enter_context(tc.tile_pool(name="consts", bufs=1))
        wcwT = cp.tile([C, C], F32)
        bcw = cp.tile([C, 1], F32)
        coefLA = cp.tile([15, C], F32)
        coefLB = cp.tile([10, C], F32)
        coefQQ = cp.tile([15, C], F32)
        coefKK = cp.tile([15, C], F32)
        cvdu = cp.tile([C, 5], F32)
        ident = cp.tile([128, 128], F32)
        ones = cp.tile([128, 128], F32)
        bdu = cp.tile([C, 1], F32)
        for t, d in [(wcwT, wcwT_d), (bcw, bcw_d), (coefLA, coefLA_d), (coefLB, coefLB_d),
                     (coefQQ, coefQQ_d), (coefKK, coefKK_d), (cvdu, cvdu_d),
                     (ident, ident_d), (ones, ones_d), (bdu, bdu_d)]:
            nc.sync.dma_start(t[:], d[:])

        xpool = ctx.enter_context(tc.tile_pool(name="xp", bufs=1))
        gfpool = ctx.enter_context(tc.tile_pool(name="gfp", bufs=1))
        gtpool = ctx.enter_context(tc.tile_pool(name="gtp", bufs=2))
        smp = ctx.enter_context(tc.tile_pool(name="smp", bufs=2))
        outp = ctx.enter_context(tc.tile_pool(name="outp", bufs=4))

        for b in range(BPC):
            xb = xpool.tile([C, NCH, 512], F32, tag="xb")
            for sl in range(4):
                nc.sync.dma_start(
                    xb[:, 8*sl:8*sl+8, :],
                    x_d[b, :, 4096*sl:4096*(sl+1)].rearrange("c (n f) -> c n f", f=512))

            GT = gtpool.tile([128, 5, 128], F32, tag="GT")   # [w, map, h]
            Gn = gtpool.tile([128, 5, 128], F32, tag="Gn")   # [g, map, w]
            nc.vector.tensor_copy(GT[:, 4, :], ones[:])

            # ---- gate, y, per-pixel channel reductions -> G^T maps ----
            with tc.tile_pool(name="s2sb", bufs=3) as sbp, \
                 tc.tile_pool(name="s2ps", bufs=2, space=bass.MemorySpace.PSUM) as psp:
                for n in range(NCH):
                    x4 = xb[:, n, :].rearrange("c (a f) -> c a f", f=128)
                    pz = psp.tile([C, 512], F32, tag="pz")
                    nc.tensor.matmul(pz[:], wcwT[:], xb[:, n, :], start=True, stop=True)
                    sig = sbp.tile([C, 4, 128], F32, tag="sig")
                    nc.scalar.activation(sig[:], pz[:].rearrange("c (a f) -> c a f", f=128),
                                         AF.Sigmoid, bias=bcw[:, 0:1])
                    ych = sbp.tile([C, 4, 128], F32, tag="ych")
                    nc.vector.scalar_tensor_tensor(ych[:], sig[:], 0.5, x4,
                                                   op0=OP.max, op1=OP.mult)
                    ptx = psp.tile([128, 4, 128], F32, tag="ptx")
                    pty = psp.tile([128, 4, 128], F32, tag="pty")
                    for j in range(4):
                        nc.tensor.transpose(ptx[:, j, :], x4[:, j, :], ident[:])
                        nc.tensor.transpose(pty[:, j, :], ych[:, j, :], ident[:])
                    nc.vector.tensor_reduce(GT[:, 0, 4*n:4*n+4], ptx[:], axis=AX.X, op=OP.add)
                    nc.vector.tensor_reduce(GT[:, 1, 4*n:4*n+4], ptx[:], axis=AX.X, op=OP.max)
                    nc.vector.tensor_reduce(GT[:, 2, 4*n:4*n+4], pty[:], axis=AX.X, op=OP.add)
                    nc.vector.tensor_reduce(GT[:, 3, 4*n:4*n+4], pty[:], axis=AX.X, op=OP.max)

            # ---- Gram maps -> gflat [25, pix]; natural G maps; norm factors ----
            gflat = gfpool.tile([15, NCH, 512], F32, tag="gflat")
            rq = smp.tile([C, 128], F32, tag="rq")
            rk = smp.tile([C, 128], F32, tag="rk")
            with tc.tile_pool(name="gsb", bufs=4) as gsb, \
                 tc.tile_pool(name="gps", bufs=2, space=bass.MemorySpace.PSUM) as gps, \
                 tc.tile_pool(name="gps1", bufs=1, space=bass.MemorySpace.PSUM) as gps1, \
                 tc.tile_pool(name="gsb1", bufs=1) as gsb1:
                gsbf = gsb1.tile([128, 15, 128], F32, tag="gsbf")
                for mi, (i, j) in enumerate(PAIRS2):
                    pg = gps.tile([128, 128], F32, tag="pg")
                    nc.tensor.matmul(pg[:], GT[:, i, :], GT[:, j, :], start=True, stop=True)
                    nc.scalar.copy(gsbf[:, mi, :], pg[:])
                nc.sync.dma_start(gdram[:].rearrange("m h g -> h m g"), gsbf[:])
                nc.scalar.dma_start(gflat[:], gdram[:].rearrange("m h (n f) -> m (h n) f", f=512)
                                    if False else gdram[:].rearrange("m h g -> m (h g)")
                                    .rearrange("m (n f) -> m n f", f=512))
                for i in range(5):
                    pgn = gps.tile([128, 128], F32, tag="pgn")
                    nc.tensor.transpose(pgn[:], GT[:, i, :], ident[:])
                    nc.scalar.copy(Gn[:, i, :], pgn[:])

                # d_m[h] = sum_w GT_i * GT_j (15 pairs) via ones-matmul dup rows
                P = gsb1.tile([128, 15, 128], F32, tag="P")
                for mi, (i, j) in enumerate(PAIRS):
                    nc.gpsimd.tensor_mul(P[:, mi, :], GT[:, i, :], GT[:, j, :])
                Pf = P[:].rearrange("w m h -> w (m h)")
                drow = gsb1.tile([1, 15, 128], F32, tag="drow")
                for q in range(4):
                    pd = gps.tile([128, 480], F32, tag="pg")
                    nc.tensor.matmul(pd[:], ones[:], Pf[:, 480*q:480*(q+1)],
                                     start=True, stop=True)
                    nc.scalar.copy(drow[:].rearrange("o m h -> o (m h)")[:, 480*q:480*(q+1)],
                                   pd[0:1, :])
                d15 = gsb1.tile([15, 128], F32, tag="d15")
                nc.sync.dma_start(d15[:],
                                  drow[:].rearrange("o m h -> o (m h)"))
                pnq = gps1.tile([C, 128], F32, tag="pnq")
                nc.tensor.matmul(pnq[:], coefQQ[:], d15[:], start=True, stop=True)
                sq = gsb.tile([C, 128], F32, tag="sq")
                nc.scalar.activation(sq[:], pnq[:], AF.Sqrt)
                nc.vector.reciprocal(rq[:], sq[:])
                pnk = gps1.tile([C, 128], F32, tag="pnk")
                nc.tensor.matmul(pnk[:], coefKK[:], d15[:], start=True, stop=True)
                sk = gsb.tile([C, 128], F32, tag="sk")
                nc.scalar.activation(sk[:], pnk[:], AF.Sqrt)
                nc.vector.reciprocal(rk[:], sk[:])

            # ---- attention chunks: L -> exp -> softmax-normalize -> M ----
            MT = gtpool.tile([128, 5, 128], F32, tag="MT")   # [g, map, h]
            with tc.tile_pool(name="asb", bufs=3) as asb, \
                 tc.tile_pool(name="msb", bufs=2) as msb, \
                 tc.tile_pool(name="aps", bufs=2, space=bass.MemorySpace.PSUM) as aps:
                for n in range(NCH):
                    pL = aps.tile([C, 512], F32, tag="pL")
                    nc.tensor.matmul(pL[:], coefLA[:], gflat[:, n, :], start=True, stop=False)
                    gvT = gflat[:].rearrange("m n f -> m (n f)") \
                                  .rearrange("m (g h) -> m h g", h=128)
                    nc.tensor.matmul(pL[:], coefLB[:], gvT[0:10, 4*n:4*n+4, :],
                                     start=False, stop=True)
                    pL4 = pL[:].rearrange("c (a f) -> c a f", f=128)
                    Ls = asb.tile([C, 4, 128], F32, tag="Ls")
                    for hh in range(4):
                        nc.vector.scalar_tensor_tensor(
                            Ls[:, hh, :], pL4[:, hh, :], rq[:, 4*n+hh:4*n+hh+1],
                            rk[:, :], op0=OP.mult, op1=OP.mult)
                    Se = asb.tile([C, 4, 128], F32, tag="Se")
                    rc = asb.tile([C, 4], F32, tag="rc")
                    for hh in range(4):
                        nc.scalar.activation(Se[:, hh, :], Ls[:, hh, :], AF.Exp,
                                             accum_out=rc[:, hh:hh+1])
                    rho = asb.tile([C, 4], F32, tag="rho")
                    nc.vector.reciprocal(rho[:], rc[:])
                    Sp = asb.tile([C, 4, 128], F32, tag="Sp")
                    for hh in range(4):
                        nc.gpsimd.tensor_scalar_mul(Sp[:, hh, :], Se[:, hh, :],
                                                    rho[:, hh:hh+1])
                    pM = aps.tile([5, 512], F32, tag="pM")
                    nc.tensor.matmul(pM[:], cvdu[:], Sp[:].rearrange("c a f -> c (a f)"),
                                     start=True, stop=True)
                    m4 = msb.tile([5, 4, 128], F32, tag="m4")
                    nc.scalar.copy(m4[:], pM[:].rearrange("p (a f) -> p a f", f=128))
                    nc.sync.dma_start(mdram[:, 4*n:4*n+4, :], m4[:])
                if True:
                    nc.sync.dma_start(MT[:], mdram[:].rearrange("n h g -> g n h"))

            # ---- du_pre = sum_n M_n^T @ G_n ; du = sigmoid ; out = x*du ----
            with tc.tile_pool(name="dsb", bufs=2) as dsb, \
                 tc.tile_pool(name="dps", bufs=2, space=bass.MemorySpace.PSUM) as dps:
                pdu = dps.tile([128, 128], F32, tag="pdu")
                for nn in range(5):
                    nc.tensor.matmul(pdu[:], MT[:, nn, :], Gn[:, nn, :],
                                     start=(nn == 0), stop=(nn == 4))
                du = dsb.tile([128, 128], F32, tag="du")
                nc.scalar.activation(du[:], pdu[:], AF.Sigmoid, bias=bdu[:, 0:1])
                durow = gfpool.tile([1, PIX], F32, tag="gflat")
                nc.gpsimd.dma_start(durow[:], du[:])
                for n in range(NCH):
                    pbc = dps.tile([128, 512], F32, tag="pbc")
                    nc.tensor.matmul(pbc[:], ones[0:1, :],
                                     durow[0:1, 512*n:512*(n+1)],
                                     start=True, stop=True)
                    if n % 4 == 0:
                        oc = outp.tile([C, 4, 512], F32, tag="oc")
                    nc.vector.tensor_mul(oc[:, n % 4, :], xb[:, n, :], pbc[:])
                    if n % 4 == 3:
                        dq = (nc.sync, nc.scalar, nc.gpsimd)[(n//4) % 3]
                        dq.dma_start(out_d[b, :, 2048*(n//4):2048*(n//4 + 1)],
                                     oc[:].rearrange("c a f -> c (a f)"))
    nc.compile()
    return nc


_NC = None


def prep_in_maps(x, w_cw, b_cw, w_qkv, b_qkv, w_du, b_du):
    x = np.asarray(x, np.float32)
    w_cw = np.asarray(w_cw, np.float32)
    b_cw = np.asarray(b_cw, np.float32)
    w_qkv = np.asarray(w_qkv, np.float32)
    b_qkv = np.asarray(b_qkv, np.float32)
    w_du = np.asarray(w_du, np.float32)
    b_du = np.asarray(b_du, np.float32)

    wq, wk, wv = w_qkv[0:C], w_qkv[C:2*C], w_qkv[2*C:3*C]
    bq, bk, bv = b_qkv[0:C], b_qkv[C:2*C], b_qkv[2*C:3*C]

    def amat(w, bias):
        return np.stack([w[:, 0]/C, w[:, 1], w[:, 2]/C, w[:, 3], bias], axis=1)

    Aq, Ak, Av = amat(wq, bq), amat(wk, bk), amat(wv, bv)
    coefLA = np.stack([Aq[:, i]*Ak[:, j] for i, j in PAIRS2]).astype(np.float32)
    coefLB = np.stack([Aq[:, j]*Ak[:, i] for i, j in PAIRS2[:10]]).astype(np.float32)
    coefQQ = np.stack([Aq[:, i]*Aq[:, j]*(1.0 if i == j else 2.0)
                       for i, j in PAIRS]).astype(np.float32)
    coefKK = np.stack([Ak[:, i]*Ak[:, j]*(1.0 if i == j else 2.0)
                       for i, j in PAIRS]).astype(np.float32)
    cvdu = (Av * w_du[0][:, None]).astype(np.float32)

    common = {
        "wcwT": np.ascontiguousarray(w_cw.T),
        "bcw": np.ascontiguousarray(b_cw[:, None]),
        "coefLA": coefLA, "coefLB": coefLB,
        "coefQQ": coefQQ, "coefKK": coefKK, "cvdu": cvdu,
        "ident": np.eye(128, dtype=np.float32),
        "ones": np.ones((128, 128), np.float32),
        "bdu": np.full((C, 1), b_du[0], np.float32),
    }
    xs = x.reshape(NCORES, BPC, C, PIX)
    return [dict(common, x=np.ascontiguousarray(xs[k])) for k in range(NCORES)]


def kernel(x, w_cw, b_cw, w_qkv, b_qkv, w_du, b_du):
    global _NC
    in_maps = prep_in_maps(x, w_cw, b_cw, w_qkv, b_qkv, w_du, b_du)
    if _NC is None:
        _NC = build_kernel()
    globals()["_LAST_IN_MAPS"] = in_maps
    res = run_bass_kernel_spmd(_NC, in_maps, core_ids=list(range(NCORES)))
    out = np.concatenate([r["out"][None] for r in res.results], axis=0)
    return out.reshape(B, C, H, W)


if __name__ == "__main__":
    import reference as R
    inp = R.setup_inputs()
    inp = {k: np.asarray(v) for k, v in inp.items()}
    got = kernel(**inp)
    exp = np.asarray(R.reference(**inp))
    err = np.abs(got - exp).max() / np.abs(exp).max()
    print("scaled absmax err:", err)
